# revision 15
# baseline (speedup 1.0000x reference)
"""Trainium2 Bass kernel for nn_E3Convolution (E3 equivariant convolution).

Strategy (8 NeuronCores, edge-parallel, zero collectives):
  - Host sorts edges by dst and cuts the sorted list at segment boundaries into
    8 contiguous shards -> each core owns a disjoint contiguous node range, so
    the scatter-add needs no cross-core reduction at all.
  - Host prep is strictly index manipulation / layout permutation / weight
    reshaping; every FLOP that scales with edges or nodes runs on device.
  - Per core: PE does all shared-weight matmuls (radial MLP, o3 linears,
    s@W pre-contractions, scatter via 0/1 matrices, transposes); DVE does the
    per-edge tensor-product contractions via stride-0-broadcast access
    patterns + in-place tree reductions; ACT does activations and strided
    copies.
"""

import sys

sys.path.insert(0, "/opt/trn_rl_repo")

import numpy as np

# ---------------- problem constants (hardcoded from the spec) ----------------
MUL0, MUL1 = 32, 16
DIM = MUL0 + 3 * MUL1            # 80
NUM_TYPE, BASIS, HIDDEN = 4, 32, 128
N_NODES, N_EDGES = 5000, 30000
N_AVG = 6.0
CAT0, CAT1 = 3 * MUL0, 3 * MUL1  # 96, 48
SQ3 = np.float32(np.sqrt(3.0))

NCORES = 8
EPC = 3840                        # padded edges per core (30 tiles of 128)
TILES = EPC // 128                # 30
NPC = 768                         # padded nodes per core (6 tiles of 128)
NT_N = NPC // 128                 # 6

F32 = None  # filled at import of mybir below


# ---------------- host-side preparation (index-only + weight reshape) --------
def _block_diag_L(L0, L1, mul0, mul1):
    """o3_linear as a single [DIM, DIM] matrix (scalars block + L1 (x) I3)."""
    d = mul0 + 3 * mul1
    M = np.zeros((d, d), np.float32)
    M[:mul0, :mul0] = L0 / np.sqrt(mul0)
    for u in range(mul1):
        for w in range(mul1):
            v = L1[u, w] / np.sqrt(mul1)
            for k in range(3):
                M[mul0 + 3 * u + k, mul0 + 3 * w + k] = v
    return M


def _reorder_wm2(Wm2):
    """Split+reorder Wm2 columns into per-path blocks with (w-outer, u-inner)
    column order, with all normalization constants folded in."""
    c = HIDDEN
    o = 0
    w_ss = Wm2[:, o:o + CAT0 * MUL0].reshape(c, CAT0, MUL0); o += CAT0 * MUL0
    w_sv = Wm2[:, o:o + CAT0 * MUL1].reshape(c, CAT0, MUL1); o += CAT0 * MUL1
    w_vs = Wm2[:, o:o + CAT1 * MUL1].reshape(c, CAT1, MUL1); o += CAT1 * MUL1
    w_vv = Wm2[:, o:].reshape(c, CAT1, MUL0)
    s = np.float32(1.0 / (np.sqrt(HIDDEN) * np.sqrt(CAT0 + CAT1)))
    # (w-outer, u-inner): dev[:, w*U + u] = w_path[:, u, w]
    dev_ss = np.ascontiguousarray(w_ss.transpose(0, 2, 1).reshape(c, -1)) * s
    dev_sv = np.ascontiguousarray(w_sv.transpose(0, 2, 1).reshape(c, -1)) * s
    dev_vs = np.ascontiguousarray(w_vs.transpose(0, 2, 1).reshape(c, -1)) * s
    dev_vv = np.ascontiguousarray(w_vv.transpose(0, 2, 1).reshape(c, -1)) * (s / SQ3)
    return dev_ss, dev_sv, dev_vs, dev_vv


def _reorder_sc(W0, W1, T):
    """sc_tp weights as [T, (w,u)] matrices for the B-form, norms folded."""
    s0 = np.float32(1.0 / np.sqrt(MUL0 * T))
    s1 = np.float32(1.0 / np.sqrt(MUL1 * T))
    # W0 [u, t, w] -> W0r[t, w*MUL0 + u]
    W0r = np.ascontiguousarray(W0.transpose(1, 2, 0).reshape(T, -1)) * s0
    W1r = np.ascontiguousarray(W1.transpose(1, 2, 0).reshape(T, -1)) * s1
    return W0r, W1r


def _shard_edges(edge_dst):
    """Sort edges by dst (stable) and cut at segment boundaries near i*E/8.
    Returns (sorted_ids, cuts[9], node_lo[9])."""
    order = np.argsort(edge_dst, kind="stable")
    dst_sorted = edge_dst[order]
    cuts = [0]
    for i in range(1, NCORES):
        t = (N_EDGES * i) // NCORES
        # move t forward to the next segment boundary (dst changes)
        while t < N_EDGES and t > 0 and dst_sorted[t] == dst_sorted[t - 1]:
            t += 1
        cuts.append(min(t, N_EDGES))
    cuts.append(N_EDGES)
    cuts = np.maximum.accumulate(np.asarray(cuts))
    node_lo = [0]
    for i in range(1, NCORES):
        a, b = cuts[i], cuts[i + 1]
        node_lo.append(int(dst_sorted[a]) if b > a else node_lo[-1])
    node_lo.append(N_NODES)
    return order, cuts, node_lo


def _host_prep(inputs):
    f_node = inputs["f_node"]; f_edge = inputs["f_edge"]
    sh = inputs["sh"]; node_emb = inputs["node_emb"]; length_emb = inputs["length_emb"]
    edge_src = inputs["edge_src"]; edge_dst = inputs["edge_dst"]

    order, cuts, node_lo = _shard_edges(edge_dst)

    # replicated (weight) tensors
    wss, wsv, wvs, wvv = _reorder_wm2(inputs["Wm2"])
    rep = {
        "Wm1s": np.ascontiguousarray(inputs["Wm1"] * np.float32(1.0 / np.sqrt(BASIS))),
        "Wss": wss, "Wsv": wsv, "Wvs": wvs, "Wvv": wvv,
        "L1n": _block_diag_L(inputs["L1n0"], inputs["L1n1"], MUL0, MUL1),
        "L1e": _block_diag_L(inputs["L1e0"], inputs["L1e1"], MUL0, MUL1),
        "L2n": _block_diag_L(inputs["L2n0"], inputs["L2n1"], MUL0, MUL1),
        "L2e": _block_diag_L(inputs["L2e0"], inputs["L2e1"], MUL0, MUL1),
    }
    rep["W0re"], rep["W1re"] = _reorder_sc(inputs["W_sce0"], inputs["W_sce1"], 2 * NUM_TYPE + BASIS)
    rep["W0rn"], rep["W1rn"] = _reorder_sc(inputs["W_scn0"], inputs["W_scn1"], NUM_TYPE)

    in_maps = []
    meta = []
    for i in range(NCORES):
        a, b = int(cuts[i]), int(cuts[i + 1])
        ln = b - a
        assert ln <= EPC, f"shard {i} too long: {ln}"
        ids = order[a:b]
        lo, hi = node_lo[i], node_lo[i + 1]
        width = hi - lo
        assert width <= NPC, f"node range {i} too wide: {width}"

        src = edge_src[ids]; dst = edge_dst[ids]

        def padE(x, fill=0.0):
            out = np.zeros((EPC,) + x.shape[1:], np.float32)
            out[:ln] = x
            return out

        fsrc = padE(f_node[src]); fdst = padE(f_node[dst]); fedg = padE(f_edge[ids])
        le = padE(length_emb[ids]); shp = padE(sh[ids])
        se = padE(np.concatenate([node_emb[src], node_emb[dst], length_emb[ids]], axis=-1))

        onehot = np.zeros((EPC, NPC), np.float32)
        onehot[np.arange(ln), dst - lo] = 1.0

        fnode_my = np.zeros((NPC, DIM), np.float32); fnode_my[:width] = f_node[lo:hi]
        nemb_my = np.zeros((NPC, NUM_TYPE), np.float32); nemb_my[:width] = node_emb[lo:hi]

        m = {
            "fsrcT": np.ascontiguousarray(fsrc.T).astype(np.float16),
            "fdstT": np.ascontiguousarray(fdst.T).astype(np.float16),
            "fedgeT": np.ascontiguousarray(fedg.T).astype(np.float16),
            "leT": np.ascontiguousarray(le.T).astype(np.float16),
            "seT": np.ascontiguousarray(se.T).astype(np.float16),
            # e-on-partition layouts [128, TILES, X]
            "fedge_p": np.ascontiguousarray(
                fedg.reshape(TILES, 128, DIM).transpose(1, 0, 2)
                .reshape(128, TILES * DIM)).astype(np.float16),
            "shp": np.ascontiguousarray(
                shp.reshape(TILES, 128, 4).transpose(1, 0, 2).reshape(128, TILES * 4)),
            "onehot": np.ascontiguousarray(onehot.reshape(TILES, 128, NPC)),
            "fnode_p": np.ascontiguousarray(
                fnode_my.reshape(NT_N, 128, DIM).transpose(1, 0, 2)
                .reshape(128, NT_N * DIM)).astype(np.float16),
            "nembT": np.ascontiguousarray(nemb_my.T).astype(np.float16),
        }
        m.update({k: (v.astype(np.float16) if k not in ("L2n", "L2e") else v)
                  for k, v in rep.items()})
        in_maps.append(m)
        meta.append((ids, ln, lo, width))
    return in_maps, meta


# ---------------- device program ---------------------------------------------
_PROG_CACHE = {}


def _chunks(total, size):
    out = []
    o = 0
    while o < total:
        c = min(size, total - o)
        out.append((o, c))
        o += c
    return out


def _build_program():
    from concourse import bass, mybir
    from concourse.tile import TileContext
    from concourse.masks import make_identity

    f32 = mybir.dt.float32
    f16 = mybir.dt.float16
    AF = mybir.ActivationFunctionType
    OP = mybir.AluOpType

    nc = bass.Bass()

    # ---- DRAM I/O ----
    F16_INPUTS = {"fsrcT", "fdstT", "fedgeT", "leT", "seT", "fedge_p", "fnode_p",
                  "nembT", "Wm1s", "Wss", "Wsv", "Wvs", "Wvv", "L1n", "L1e",
                  "W0re", "W1re", "W0rn", "W1rn"}
    D = {}
    def din(name, shape):
        dt = f16 if name in F16_INPUTS else f32
        D[name] = nc.dram_tensor(name, list(shape), dt, kind="ExternalInput")
    for nm, shp in [
        ("fsrcT", (DIM, EPC)), ("fdstT", (DIM, EPC)), ("fedgeT", (DIM, EPC)),
        ("leT", (BASIS, EPC)), ("seT", (2 * NUM_TYPE + BASIS, EPC)),
        ("fedge_p", (128, TILES * DIM)), ("shp", (128, TILES * 4)),
        ("onehot", (TILES, 128, NPC)),
        ("fnode_p", (128, NT_N * DIM)), ("nembT", (NUM_TYPE, NPC)),
        ("Wm1s", (BASIS, HIDDEN)),
        ("Wss", (HIDDEN, MUL0 * CAT0)), ("Wsv", (HIDDEN, MUL1 * CAT0)),
        ("Wvs", (HIDDEN, MUL1 * CAT1)), ("Wvv", (HIDDEN, MUL0 * CAT1)),
        ("L1n", (DIM, DIM)), ("L1e", (DIM, DIM)), ("L2n", (DIM, DIM)), ("L2e", (DIM, DIM)),
        ("W0re", (2 * NUM_TYPE + BASIS, MUL0 * MUL0)),
        ("W1re", (2 * NUM_TYPE + BASIS, MUL1 * MUL1)),
        ("W0rn", (NUM_TYPE, MUL0 * MUL0)), ("W1rn", (NUM_TYPE, MUL1 * MUL1)),
    ]:
        din(nm, shp)
    feT_out = nc.dram_tensor("feT_out", [DIM, EPC], f32, kind="ExternalOutput")
    fn_out = nc.dram_tensor("fn_out", [128, NT_N * DIM], f32, kind="ExternalOutput")

    with TileContext(nc) as tc:
        with (
            tc.tile_pool(name="const", bufs=1) as cst,
            tc.tile_pool(name="work", bufs=2) as wrk,
            tc.tile_pool(name="prod", bufs=2) as prd,
            tc.tile_pool(name="oh", bufs=3) as ohp,
            tc.tile_pool(name="outp", bufs=3) as outp,
            tc.tile_pool(name="pacc", bufs=1, space="PSUM") as pacc,
            tc.tile_pool(name="pw", bufs=4, space="PSUM") as pwp,
            tc.tile_pool(name="pmisc", bufs=1, space="PSUM") as pmp,
        ):
            # ---- load constants / per-core resident arrays ----
            def load(name):
                t = cst.tile(list(D[name].shape), D[name].dtype, tag=name)
                nc.gpsimd.dma_start(t[:], D[name][:])
                return t
            ident = cst.tile([128, 128], f32, tag="ident")
            make_identity(nc, ident[:])
            wm1 = load("Wm1s"); leT = load("leT")
            fsrcT = load("fsrcT"); fdstT = load("fdstT"); fedgeT = load("fedgeT")
            l1n = load("L1n"); l1e = load("L1e"); shp = load("shp")
            wss = load("Wss"); wvv = load("Wvv"); wsv = load("Wsv"); wvs = load("Wvs")
            seT = load("seT"); fedge_p = load("fedge_p")
            w0re = load("W0re"); w1re = load("W1re")
            l2n = load("L2n"); l2e = load("L2e")
            w0rn = load("W0rn"); w1rn = load("W1rn")
            fnode_p = load("fnode_p"); nembT = load("nembT")

            # ---- phase 1: hT = silu(Wm1s.T @ leT)  [HIDDEN, EPC] ----
            hsb = cst.tile([128, EPC], f16, tag="hsb")
            for (o, n) in _chunks(EPC, 512):
                ph = pwp.tile([128, 512], f32, tag="pw")
                nc.tensor.matmul(ph[:, :n], wm1[:], leT[:, o:o + n], start=True, stop=True)
                nc.scalar.activation(hsb[:, o:o + n], ph[:, :n], AF.Silu)

            # persistent scatter accumulator [DIM, NPC] (2 psum banks)
            fnT_acc = pacc.tile([DIM, NPC], f32)

            def mm2_block(lhsT, wmat, total, ve_tag, scale=None):
                """matmul lhsT.T @ wmat[:, :total] in 512-col psum chunks,
                ACT-evac (optionally scaled) into one f16 SBUF tile."""
                ve = wrk.tile([128, max(total, 512)], f16, tag=ve_tag)
                for (o2, n2) in _chunks(total, 512):
                    pw = pwp.tile([128, 512], f32, tag="pw")
                    nc.tensor.matmul(pw[:, :n2], lhsT, wmat[:, o2:o2 + n2],
                                     start=True, stop=True)
                    if scale is None:
                        nc.scalar.copy(ve[:, o2:o2 + n2], pw[:, :n2])
                    else:
                        nc.scalar.mul(ve[:, o2:o2 + n2], pw[:, :n2], scale)
                return ve


            # helper: in-place pairwise tree over innermost dim, then reduce
            def tree(P, nw, u, tag):
                """P: AP view [128, nw, u] (SBUF). Returns [128, nw] tile."""
                while u > 3 and u % 2 == 0:
                    h = u // 2
                    nc.vector.tensor_add(P[:, :, 0:h], P[:, :, 0:h], P[:, :, h:u])
                    u = h
                r = prd.tile([128, nw], f32, tag=tag + "_r")
                nc.vector.tensor_reduce(r[:], P[:, :, 0:u], axis=mybir.AxisListType.X,
                                        op=OP.add)
                return r

            # ---- phase 2: edge tiles ----
            for t in range(TILES):
                sl = slice(t * 128, (t + 1) * 128)
                sh0 = shp[:, 4 * t:4 * t + 1]
                sh1 = shp[:, 4 * t + 1:4 * t + 4]

                # o3 linears for the three cat sources -> one psum bank
                po3 = pmp.tile([128, 256], f32, tag="po3")  # 1 bank
                nc.tensor.matmul(po3[:, 0:80], fsrcT[:, sl], l1n[:], start=True, stop=True)
                nc.tensor.matmul(po3[:, 80:160], fdstT[:, sl], l1n[:], start=True, stop=True)
                nc.tensor.matmul(po3[:, 160:240], fedgeT[:, sl], l1e[:], start=True, stop=True)

                # assemble cat0 [128,96], cat1r [128,(3k,48u)]
                cat0 = wrk.tile([128, CAT0], f16, tag="cat0")
                cat1r = wrk.tile([128, 3 * CAT1], f16, tag="cat1r")
                c1v = cat1r[:].rearrange("p (k u) -> p k u", k=3)
                for j in range(3):
                    base = 80 * j
                    nc.scalar.copy(cat0[:, 32 * j:32 * (j + 1)], po3[:, base:base + 32])
                    src = po3[:, base + 32:base + 80].rearrange("p (u k) -> p k u", k=3)
                    nc.scalar.copy(c1v[:, :, 16 * j:16 * (j + 1)], src)

                # F2[u] = sum_k cat1[u,k]*sh1[k]  (vv input)
                f2a = wrk.tile([128, CAT1], f16, tag="f2a")
                f2 = wrk.tile([128, CAT1], f16, tag="f2")
                nc.vector.tensor_scalar_mul(f2a[:], cat1r[:, 0:48], sh1[:, 0:1])
                nc.vector.scalar_tensor_tensor(out=f2[:], in0=cat1r[:, 48:96],
                                               scalar=sh1[:, 1:2], in1=f2a[:],
                                               op0=OP.mult, op1=OP.add)
                nc.vector.scalar_tensor_tensor(out=f2[:], in0=cat1r[:, 96:144],
                                               scalar=sh1[:, 2:3], in1=f2[:],
                                               op0=OP.mult, op1=OP.add)

                # ---- mm2 + per-edge TP products ----
                # SS block: cols (32w x 96u); products scaled by sh0
                PA = prd.tile([128, (MUL0 + MUL1) * CAT0], f16, tag="PA")
                pav = PA[:].rearrange("p (g u) -> p g u", u=CAT0)
                ve_ss = mm2_block(hsb[:, sl], wss[:], MUL0 * CAT0, "ve_ss", scale=sh0)
                nc.vector.tensor_tensor(
                    out=pav[:, 0:MUL0, :],
                    in0=ve_ss[:, :MUL0 * CAT0].rearrange("p (w u) -> p w u", u=CAT0),
                    in1=cat0[:].rearrange("p (o_ u) -> p o_ u", o_=1)
                        .broadcast_to([128, MUL0, CAT0]),
                    op=OP.mult)

                # VV block: cols (32w x 48u); in1 = F2 (already has sh1 folded)
                PB = prd.tile([128, (MUL0 + 3 * MUL1) * CAT1], f16, tag="PB")
                pbv = PB[:].rearrange("p (g u) -> p g u", u=CAT1)
                ve_vv = mm2_block(hsb[:, sl], wvv[:], MUL0 * CAT1, "ve_vv")
                nc.vector.tensor_tensor(
                    out=pbv[:, 0:MUL0, :],
                    in0=ve_vv[:, :MUL0 * CAT1].rearrange("p (w u) -> p w u", u=CAT1),
                    in1=f2[:].rearrange("p (o_ u) -> p o_ u", o_=1)
                        .broadcast_to([128, MUL0, CAT1]),
                    op=OP.mult)

                # SV block: cols (16w x 96u); t16[w] = sum_u cat0[u]*w_sv
                ve_sv = mm2_block(hsb[:, sl], wsv[:], MUL1 * CAT0, "ve_sv")
                nc.vector.tensor_tensor(
                    out=pav[:, MUL0:MUL0 + MUL1, :],
                    in0=ve_sv[:, :MUL1 * CAT0].rearrange("p (w u) -> p w u", u=CAT0),
                    in1=cat0[:].rearrange("p (o_ u) -> p o_ u", o_=1)
                        .broadcast_to([128, MUL1, CAT0]),
                    op=OP.mult)

                # VS block: cols (16w x 48u); shared over k, scaled by sh0
                pvsv = pbv[:, MUL0:, :].rearrange("p (k w) u -> p k w u", k=3)
                ve_vs = mm2_block(hsb[:, sl], wvs[:], MUL1 * CAT1, "ve_vs", scale=sh0)
                iv = ve_vs[:, :MUL1 * CAT1].rearrange("p (w u) -> p w u", u=CAT1)
                for k in range(3):
                    bc = cat1r[:, k * CAT1:(k + 1) * CAT1] \
                        .rearrange("p (o_ u) -> p o_ u", o_=1).broadcast_to([128, MUL1, CAT1])
                    nc.vector.tensor_tensor(out=pvsv[:, k, :, :], in0=iv, in1=bc, op=OP.mult)
                # fused trees over PA (u=96) and PB (u=48)
                rA = tree(pav, MUL0 + MUL1, CAT0, "rA")          # [128, 48]: y0a | t16
                rB = tree(pbv, MUL0 + 3 * MUL1, CAT1, "rB")      # [128, 80]: y0b | vs48(k,w)
                y0a = rA[:, 0:MUL0]; t16v = rA[:, MUL0:MUL0 + MUL1]
                y0b = rB[:, 0:MUL0]; vs48v = rB[:, MUL0:MUL0 + 3 * MUL1]

                # y0 = silu(y0a + y0b) -> fe_gated[:, :32]
                fe_g = outp.tile([128, DIM], f32, tag="fe_g")
                y0 = wrk.tile([128, MUL0], f32, tag="y0")
                nc.vector.tensor_add(y0[:], y0a, y0b)
                nc.scalar.activation(fe_g[:, 0:MUL0], y0[:], AF.Silu)

                # y1[(w,k)] = t16[w]*sh1[k] + vs48[(k,w)]
                y1 = wrk.tile([128, 3 * MUL1], f32, tag="y1")
                y1v = y1[:].rearrange("p (w k) -> p w k", k=3)
                t16b = t16v.rearrange("p (w o_) -> p w o_", o_=1).broadcast_to([128, MUL1, 3])
                sh1b = sh1[:].rearrange("p (o_ k) -> p o_ k", o_=1).broadcast_to([128, MUL1, 3])
                nc.vector.tensor_tensor(out=y1v, in0=t16b, in1=sh1b, op=OP.mult)
                vsv = vs48v.rearrange("p (k w) -> p w k", k=3)
                nc.vector.tensor_add(y1v, y1v, vsv)

                # gate: sigmoid(|y1|) per vector
                sq = wrk.tile([128, 3 * MUL1], f32, tag="sq")
                nc.scalar.activation(sq[:], y1[:], AF.Square)
                n2 = wrk.tile([128, MUL1], f32, tag="n2")
                nc.vector.tensor_reduce(n2[:], sq[:].rearrange("p (w k) -> p w k", k=3),
                                        axis=mybir.AxisListType.X, op=OP.add)
                nrm = wrk.tile([128, MUL1], f32, tag="nrm")
                nc.scalar.activation(nrm[:], n2[:], AF.Sqrt)
                gsig = wrk.tile([128, MUL1], f32, tag="gsig")
                nc.scalar.activation(gsig[:], nrm[:], AF.Sigmoid)
                gb = gsig[:].rearrange("p (w o_) -> p w o_", o_=1).broadcast_to([128, MUL1, 3])
                fgv = fe_g[:, MUL0:DIM].rearrange("p (w k) -> p w k", k=3)
                nc.vector.tensor_tensor(out=fgv, in0=y1v, in1=gb, op=OP.mult)

                # ---- sc_edge (B-form) ----
                x0e = fedge_p[:, t * DIM:t * DIM + MUL0]
                x1r = wrk.tile([128, 3 * MUL1], f32, tag="x1r")
                nc.scalar.copy(
                    x1r[:].rearrange("p (k u) -> p k u", k=3),
                    fedge_p[:, t * DIM + MUL0:(t + 1) * DIM].rearrange("p (u k) -> p k u", k=3))

                Pb0 = prd.tile([128, MUL0 * MUL0], f32, tag="Pb0")
                for (o, n) in _chunks(MUL0 * MUL0, 512):
                    nw = n // MUL0
                    pw = pwp.tile([128, 512], f32, tag="pw")
                    nc.tensor.matmul(pw[:, :n], seT[:, sl], w0re[:, o:o + n],
                                     start=True, stop=True)
                    ov = Pb0[:, o:o + n].rearrange("p (w u) -> p w u", u=MUL0)
                    iv = pw[:, :n].rearrange("p (w u) -> p w u", u=MUL0)
                    bc = x0e.rearrange("p (o_ u) -> p o_ u", o_=1).broadcast_to([128, nw, MUL0])
                    nc.vector.tensor_tensor(out=ov, in0=iv, in1=bc, op=OP.mult)
                sc0 = tree(Pb0[:].rearrange("p (w u) -> p w u", u=MUL0), MUL0, MUL0, "sc0")

                Pb1 = prd.tile([128, 3 * MUL1 * MUL1], f32, tag="Pb1")
                pb1v = Pb1[:].rearrange("p (k w u) -> p k w u", k=3, u=MUL1)
                pw = pwp.tile([128, 512], f32, tag="pw")
                nc.tensor.matmul(pw[:, :MUL1 * MUL1], seT[:, sl], w1re[:],
                                 start=True, stop=True)
                iv = pw[:, :MUL1 * MUL1].rearrange("p (w u) -> p w u", u=MUL1)
                for k in range(3):
                    bc = x1r[:, k * MUL1:(k + 1) * MUL1] \
                        .rearrange("p (o_ u) -> p o_ u", o_=1).broadcast_to([128, MUL1, MUL1])
                    nc.vector.tensor_tensor(out=pb1v[:, k, :, :], in0=iv, in1=bc, op=OP.mult)
                sc1 = tree(Pb1[:].rearrange("p (g u) -> p g u", u=MUL1), 3 * MUL1, MUL1, "sc1")

                sc_e = outp.tile([128, DIM], f32, tag="sc_e")
                nc.scalar.copy(sc_e[:, 0:MUL0], sc0[:])
                nc.scalar.copy(
                    sc_e[:, MUL0:DIM].rearrange("p (w k) -> p w k", k=3),
                    sc1[:].rearrange("p (k w) -> p w k", k=3))

                # ---- scatter-add into fnT_acc via 0/1 matmul ----
                oh = ohp.tile([128, NPC], f32, tag="oh")
                nc.gpsimd.dma_start(oh[:], D["onehot"][t])
                nc.tensor.matmul(fnT_acc[:, 0:512], fe_g[:], oh[:, 0:512],
                                 start=(t == 0), stop=(t == TILES - 1), skip_group_check=True)
                nc.tensor.matmul(fnT_acc[:, 512:NPC], fe_g[:], oh[:, 512:NPC],
                                 start=(t == 0), stop=(t == TILES - 1), skip_group_check=True)

                # ---- fe output: (gate @ L2e + sc_e)^T ----
                ptp = pmp.tile([DIM, 384], f32, tag="ptp")  # 1 bank
                nc.tensor.transpose(ptp[:, 0:128], fe_g[:], ident[:])
                nc.tensor.transpose(ptp[:, 128:256], sc_e[:], ident[:])
                geT = wrk.tile([DIM, 128], f32, tag="geT")
                nc.scalar.copy(geT[:], ptp[:, 0:128])
                scT = wrk.tile([DIM, 128], f32, tag="scT")
                nc.scalar.copy(scT[:], ptp[:, 128:256])
                nc.tensor.matmul(ptp[:, 256:384], l2e[:], geT[:], start=True, stop=True)
                feT_t = outp.tile([DIM, 128], f32, tag="feT_t")
                nc.vector.tensor_add(feT_t[:], ptp[:, 256:384], scT[:])
                nc.gpsimd.dma_start(feT_out[:, sl], feT_t[:])

            # ---- phase 3: node outputs ----
            fnT_sb = cst.tile([DIM, NPC], f32, tag="fnT_sb")
            nc.scalar.mul(fnT_sb[:], fnT_acc[:], 1.0 / N_AVG)
            for nt in range(NT_N):
                nsl = slice(nt * 128, (nt + 1) * 128)
                pl2n = pmp.tile([128, 256], f32, tag="po3")  # 1 bank
                nc.tensor.matmul(pl2n[:, 0:DIM], fnT_sb[:, nsl], l2n[:], start=True, stop=True)

                x0n = fnode_p[:, nt * DIM:nt * DIM + MUL0]
                x1rn = wrk.tile([128, 3 * MUL1], f32, tag="x1r")
                nc.scalar.copy(
                    x1rn[:].rearrange("p (k u) -> p k u", k=3),
                    fnode_p[:, nt * DIM + MUL0:(nt + 1) * DIM].rearrange("p (u k) -> p k u", k=3))

                Pb0 = prd.tile([128, MUL0 * MUL0], f32, tag="Pb0")
                for (o, n) in _chunks(MUL0 * MUL0, 512):
                    nw = n // MUL0
                    pw = pwp.tile([128, 512], f32, tag="pw")
                    nc.tensor.matmul(pw[:, :n], nembT[:, nsl], w0rn[:, o:o + n],
                                     start=True, stop=True)
                    ov = Pb0[:, o:o + n].rearrange("p (w u) -> p w u", u=MUL0)
                    iv = pw[:, :n].rearrange("p (w u) -> p w u", u=MUL0)
                    bc = x0n.rearrange("p (o_ u) -> p o_ u", o_=1).broadcast_to([128, nw, MUL0])
                    nc.vector.tensor_tensor(out=ov, in0=iv, in1=bc, op=OP.mult)
                sc0 = tree(Pb0[:].rearrange("p (w u) -> p w u", u=MUL0), MUL0, MUL0, "sc0")

                Pb1 = prd.tile([128, 3 * MUL1 * MUL1], f32, tag="Pb1")
                pb1v = Pb1[:].rearrange("p (k w u) -> p k w u", k=3, u=MUL1)
                pw = pwp.tile([128, 512], f32, tag="pw")
                nc.tensor.matmul(pw[:, :MUL1 * MUL1], nembT[:, nsl], w1rn[:],
                                 start=True, stop=True)
                iv = pw[:, :MUL1 * MUL1].rearrange("p (w u) -> p w u", u=MUL1)
                for k in range(3):
                    bc = x1rn[:, k * MUL1:(k + 1) * MUL1] \
                        .rearrange("p (o_ u) -> p o_ u", o_=1).broadcast_to([128, MUL1, MUL1])
                    nc.vector.tensor_tensor(out=pb1v[:, k, :, :], in0=iv, in1=bc, op=OP.mult)
                sc1 = tree(Pb1[:].rearrange("p (g u) -> p g u", u=MUL1), 3 * MUL1, MUL1, "sc1")

                fn_t = outp.tile([128, DIM], f32, tag="fn_t")
                nc.vector.tensor_add(fn_t[:, 0:MUL0], pl2n[:, 0:MUL0], sc0[:])
                nc.vector.tensor_add(
                    fn_t[:, MUL0:DIM].rearrange("p (w k) -> p w k", k=3),
                    pl2n[:, MUL0:DIM].rearrange("p (w k) -> p w k", k=3),
                    sc1[:].rearrange("p (k w) -> p w k", k=3))
                nc.gpsimd.dma_start(fn_out[:, nt * DIM:(nt + 1) * DIM], fn_t[:])

    import bass_rust as _bass_rust
    _bass_rust.move_matmul_waits_to_ldweights(nc.m)
    _bass_rust.generate_event_semaphores(nc)
    return nc


def _get_program():
    if "nc" not in _PROG_CACHE:
        _PROG_CACHE["nc"] = _build_program()
    return _PROG_CACHE["nc"]


# ---------------- entry point -------------------------------------------------
def kernel(**inputs):
    inputs = {k: np.asarray(v) for k, v in inputs.items()}
    in_maps, meta = _host_prep(inputs)
    nc = _get_program()

    from concourse.bass_utils import run_bass_kernel_spmd
    res = run_bass_kernel_spmd(nc, in_maps, list(range(NCORES)))
    _PROG_CACHE["last_results"] = res

    fn = np.zeros((N_NODES, DIM), np.float32)
    fe = np.zeros((N_EDGES, DIM), np.float32)
    for i in range(NCORES):
        ids, ln, lo, width = meta[i]
        r = res.results[i]
        fn_my = r["fn_out"].reshape(128, NT_N, DIM).transpose(1, 0, 2).reshape(NPC, DIM)
        fn[lo:lo + width] = fn_my[:width]
        fe[ids] = r["feT_out"].T[:ln]
    return fn, fe


# revision 16
# speedup vs baseline: 1.0203x; 1.0203x over previous
"""Trainium2 Bass kernel for nn_E3Convolution (E3 equivariant convolution).

Strategy (8 NeuronCores, edge-parallel, zero collectives):
  - Host sorts edges by dst and cuts the sorted list at segment boundaries into
    8 contiguous shards -> each core owns a disjoint contiguous node range, so
    the scatter-add needs no cross-core reduction at all.
  - Host prep is strictly index manipulation / layout permutation / weight
    reshaping; every FLOP that scales with edges or nodes runs on device.
  - Per core: PE does all shared-weight matmuls (radial MLP, o3 linears,
    s@W pre-contractions, scatter via 0/1 matrices, transposes); DVE does the
    per-edge tensor-product contractions via stride-0-broadcast access
    patterns + in-place tree reductions; ACT does activations and strided
    copies.
"""

import sys

sys.path.insert(0, "/opt/trn_rl_repo")

import numpy as np

# ---------------- problem constants (hardcoded from the spec) ----------------
MUL0, MUL1 = 32, 16
DIM = MUL0 + 3 * MUL1            # 80
NUM_TYPE, BASIS, HIDDEN = 4, 32, 128
N_NODES, N_EDGES = 5000, 30000
N_AVG = 6.0
CAT0, CAT1 = 3 * MUL0, 3 * MUL1  # 96, 48
SQ3 = np.float32(np.sqrt(3.0))

NCORES = 8
EPC = 3840                        # padded edges per core (30 tiles of 128)
TILES = EPC // 128                # 30
NPC = 768                         # padded nodes per core (6 tiles of 128)
NT_N = NPC // 128                 # 6

F32 = None  # filled at import of mybir below


# ---------------- host-side preparation (index-only + weight reshape) --------
def _block_diag_L(L0, L1, mul0, mul1):
    """o3_linear as a single [DIM, DIM] matrix (scalars block + L1 (x) I3)."""
    d = mul0 + 3 * mul1
    M = np.zeros((d, d), np.float32)
    M[:mul0, :mul0] = L0 / np.sqrt(mul0)
    for u in range(mul1):
        for w in range(mul1):
            v = L1[u, w] / np.sqrt(mul1)
            for k in range(3):
                M[mul0 + 3 * u + k, mul0 + 3 * w + k] = v
    return M


def _reorder_wm2(Wm2):
    """Split+reorder Wm2 columns into per-path blocks with (w-outer, u-inner)
    column order, with all normalization constants folded in."""
    c = HIDDEN
    o = 0
    w_ss = Wm2[:, o:o + CAT0 * MUL0].reshape(c, CAT0, MUL0); o += CAT0 * MUL0
    w_sv = Wm2[:, o:o + CAT0 * MUL1].reshape(c, CAT0, MUL1); o += CAT0 * MUL1
    w_vs = Wm2[:, o:o + CAT1 * MUL1].reshape(c, CAT1, MUL1); o += CAT1 * MUL1
    w_vv = Wm2[:, o:].reshape(c, CAT1, MUL0)
    s = np.float32(1.0 / (np.sqrt(HIDDEN) * np.sqrt(CAT0 + CAT1)))
    # (w-outer, u-inner): dev[:, w*U + u] = w_path[:, u, w]
    dev_ss = np.ascontiguousarray(w_ss.transpose(0, 2, 1).reshape(c, -1)) * s
    dev_sv = np.ascontiguousarray(w_sv.transpose(0, 2, 1).reshape(c, -1)) * s
    dev_vs = np.ascontiguousarray(w_vs.transpose(0, 2, 1).reshape(c, -1)) * s
    dev_vv = np.ascontiguousarray(w_vv.transpose(0, 2, 1).reshape(c, -1)) * (s / SQ3)
    return dev_ss, dev_sv, dev_vs, dev_vv


def _reorder_sc(W0, W1, T):
    """sc_tp weights as [T, (w,u)] matrices for the B-form, norms folded."""
    s0 = np.float32(1.0 / np.sqrt(MUL0 * T))
    s1 = np.float32(1.0 / np.sqrt(MUL1 * T))
    # W0 [u, t, w] -> W0r[t, w*MUL0 + u]
    W0r = np.ascontiguousarray(W0.transpose(1, 2, 0).reshape(T, -1)) * s0
    W1r = np.ascontiguousarray(W1.transpose(1, 2, 0).reshape(T, -1)) * s1
    return W0r, W1r


def _shard_edges(edge_dst):
    """Sort edges by dst (stable) and cut at segment boundaries near i*E/8.
    Returns (sorted_ids, cuts[9], node_lo[9])."""
    order = np.argsort(edge_dst, kind="stable")
    dst_sorted = edge_dst[order]
    cuts = [0]
    for i in range(1, NCORES):
        t = (N_EDGES * i) // NCORES
        # move t forward to the next segment boundary (dst changes)
        while t < N_EDGES and t > 0 and dst_sorted[t] == dst_sorted[t - 1]:
            t += 1
        cuts.append(min(t, N_EDGES))
    cuts.append(N_EDGES)
    cuts = np.maximum.accumulate(np.asarray(cuts))
    node_lo = [0]
    for i in range(1, NCORES):
        a, b = cuts[i], cuts[i + 1]
        node_lo.append(int(dst_sorted[a]) if b > a else node_lo[-1])
    node_lo.append(N_NODES)
    return order, cuts, node_lo


def _host_prep(inputs):
    f_node = inputs["f_node"]; f_edge = inputs["f_edge"]
    sh = inputs["sh"]; node_emb = inputs["node_emb"]; length_emb = inputs["length_emb"]
    edge_src = inputs["edge_src"]; edge_dst = inputs["edge_dst"]

    order, cuts, node_lo = _shard_edges(edge_dst)

    # replicated (weight) tensors
    wss, wsv, wvs, wvv = _reorder_wm2(inputs["Wm2"])
    rep = {
        "Wm1s": np.ascontiguousarray(inputs["Wm1"] * np.float32(1.0 / np.sqrt(BASIS))),
        "Wss": wss, "Wsv": wsv, "Wvs": wvs, "Wvv": wvv,
        "L1n": _block_diag_L(inputs["L1n0"], inputs["L1n1"], MUL0, MUL1),
        "L1e": _block_diag_L(inputs["L1e0"], inputs["L1e1"], MUL0, MUL1),
        "L2n": _block_diag_L(inputs["L2n0"], inputs["L2n1"], MUL0, MUL1),
        "L2e": _block_diag_L(inputs["L2e0"], inputs["L2e1"], MUL0, MUL1),
    }
    rep["W0re"], rep["W1re"] = _reorder_sc(inputs["W_sce0"], inputs["W_sce1"], 2 * NUM_TYPE + BASIS)
    rep["W0rn"], rep["W1rn"] = _reorder_sc(inputs["W_scn0"], inputs["W_scn1"], NUM_TYPE)

    in_maps = []
    meta = []
    for i in range(NCORES):
        a, b = int(cuts[i]), int(cuts[i + 1])
        ln = b - a
        assert ln <= EPC, f"shard {i} too long: {ln}"
        ids = order[a:b]
        lo, hi = node_lo[i], node_lo[i + 1]
        width = hi - lo
        assert width <= NPC, f"node range {i} too wide: {width}"

        src = edge_src[ids]; dst = edge_dst[ids]

        def padE(x, fill=0.0):
            out = np.zeros((EPC,) + x.shape[1:], np.float32)
            out[:ln] = x
            return out

        fsrc = padE(f_node[src]); fdst = padE(f_node[dst]); fedg = padE(f_edge[ids])
        le = padE(length_emb[ids]); shp = padE(sh[ids])
        se = padE(np.concatenate([node_emb[src], node_emb[dst], length_emb[ids]], axis=-1))

        onehot = np.zeros((EPC, NPC), np.float32)
        onehot[np.arange(ln), dst - lo] = 1.0

        fnode_my = np.zeros((NPC, DIM), np.float32); fnode_my[:width] = f_node[lo:hi]
        nemb_my = np.zeros((NPC, NUM_TYPE), np.float32); nemb_my[:width] = node_emb[lo:hi]

        m = {
            "fsrcT": np.ascontiguousarray(fsrc.T).astype(np.float16),
            "fdstT": np.ascontiguousarray(fdst.T).astype(np.float16),
            "fedgeT": np.ascontiguousarray(fedg.T).astype(np.float16),
            "leT": np.ascontiguousarray(le.T).astype(np.float16),
            "seT": np.ascontiguousarray(se.T).astype(np.float16),
            # e-on-partition layouts [128, TILES, X]
            "fedge_p": np.ascontiguousarray(
                fedg.reshape(TILES, 128, DIM).transpose(1, 0, 2)
                .reshape(128, TILES * DIM)).astype(np.float16),
            "shp": np.ascontiguousarray(
                shp.reshape(TILES, 128, 4).transpose(1, 0, 2).reshape(128, TILES * 4)),
            "onehot": np.ascontiguousarray(onehot.reshape(TILES, 128, NPC)),
            "fnode_p": np.ascontiguousarray(
                fnode_my.reshape(NT_N, 128, DIM).transpose(1, 0, 2)
                .reshape(128, NT_N * DIM)).astype(np.float16),
            "nembT": np.ascontiguousarray(nemb_my.T).astype(np.float16),
        }
        m.update({k: (v.astype(np.float16) if k not in ("L2n", "L2e") else v)
                  for k, v in rep.items()})
        in_maps.append(m)
        meta.append((ids, ln, lo, width))
    return in_maps, meta


# ---------------- device program ---------------------------------------------
_PROG_CACHE = {}


def _chunks(total, size):
    out = []
    o = 0
    while o < total:
        c = min(size, total - o)
        out.append((o, c))
        o += c
    return out


def _build_program():
    from concourse import bass, mybir
    from concourse.tile import TileContext
    from concourse.masks import make_identity

    f32 = mybir.dt.float32
    f16 = mybir.dt.float16
    AF = mybir.ActivationFunctionType
    OP = mybir.AluOpType

    nc = bass.Bass()

    # ---- DRAM I/O ----
    F16_INPUTS = {"fsrcT", "fdstT", "fedgeT", "leT", "seT", "fedge_p", "fnode_p",
                  "nembT", "Wm1s", "Wss", "Wsv", "Wvs", "Wvv", "L1n", "L1e",
                  "W0re", "W1re", "W0rn", "W1rn"}
    D = {}
    def din(name, shape):
        dt = f16 if name in F16_INPUTS else f32
        D[name] = nc.dram_tensor(name, list(shape), dt, kind="ExternalInput")
    for nm, shp in [
        ("fsrcT", (DIM, EPC)), ("fdstT", (DIM, EPC)), ("fedgeT", (DIM, EPC)),
        ("leT", (BASIS, EPC)), ("seT", (2 * NUM_TYPE + BASIS, EPC)),
        ("fedge_p", (128, TILES * DIM)), ("shp", (128, TILES * 4)),
        ("onehot", (TILES, 128, NPC)),
        ("fnode_p", (128, NT_N * DIM)), ("nembT", (NUM_TYPE, NPC)),
        ("Wm1s", (BASIS, HIDDEN)),
        ("Wss", (HIDDEN, MUL0 * CAT0)), ("Wsv", (HIDDEN, MUL1 * CAT0)),
        ("Wvs", (HIDDEN, MUL1 * CAT1)), ("Wvv", (HIDDEN, MUL0 * CAT1)),
        ("L1n", (DIM, DIM)), ("L1e", (DIM, DIM)), ("L2n", (DIM, DIM)), ("L2e", (DIM, DIM)),
        ("W0re", (2 * NUM_TYPE + BASIS, MUL0 * MUL0)),
        ("W1re", (2 * NUM_TYPE + BASIS, MUL1 * MUL1)),
        ("W0rn", (NUM_TYPE, MUL0 * MUL0)), ("W1rn", (NUM_TYPE, MUL1 * MUL1)),
    ]:
        din(nm, shp)
    feT_out = nc.dram_tensor("feT_out", [DIM, EPC], f32, kind="ExternalOutput")
    fn_out = nc.dram_tensor("fn_out", [128, NT_N * DIM], f32, kind="ExternalOutput")

    with TileContext(nc) as tc:
        with (
            tc.tile_pool(name="const", bufs=1) as cst,
            tc.tile_pool(name="work", bufs=2) as wrk,
            tc.tile_pool(name="prod", bufs=2) as prd,
            tc.tile_pool(name="oh", bufs=3) as ohp,
            tc.tile_pool(name="outp", bufs=3) as outp,
            tc.tile_pool(name="pacc", bufs=1, space="PSUM") as pacc,
            tc.tile_pool(name="pw", bufs=4, space="PSUM") as pwp,
            tc.tile_pool(name="pmisc", bufs=1, space="PSUM") as pmp,
        ):
            # ---- load constants / per-core resident arrays ----
            def load(name, split=1):
                t = cst.tile(list(D[name].shape), D[name].dtype, tag=name)
                cols = D[name].shape[-1]
                step = (cols + split - 1) // split
                for (o, n) in _chunks(cols, step):
                    nc.gpsimd.dma_start(t[:, o:o + n], D[name][:, o:o + n])
                return t
            ident = cst.tile([128, 128], f32, tag="ident")
            make_identity(nc, ident[:])
            wm1 = load("Wm1s"); leT = load("leT", split=4)
            fsrcT = load("fsrcT", split=4); fdstT = load("fdstT", split=4)
            fedgeT = load("fedgeT", split=4)
            l1n = load("L1n"); l1e = load("L1e"); shp = load("shp")
            wss = load("Wss", split=2); wvv = load("Wvv"); wsv = load("Wsv")
            wvs = load("Wvs")
            seT = load("seT", split=4); fedge_p = load("fedge_p", split=4)
            w0re = load("W0re"); w1re = load("W1re")
            l2n = load("L2n"); l2e = load("L2e")
            w0rn = load("W0rn"); w1rn = load("W1rn")
            fnode_p = load("fnode_p"); nembT = load("nembT")

            # ---- phase 1: hT = silu(Wm1s.T @ leT)  [HIDDEN, EPC] ----
            hsb = cst.tile([128, EPC], f16, tag="hsb")
            for (o, n) in _chunks(EPC, 512):
                ph = pwp.tile([128, 512], f32, tag="pw")
                nc.tensor.matmul(ph[:, :n], wm1[:], leT[:, o:o + n], start=True, stop=True)
                nc.scalar.activation(hsb[:, o:o + n], ph[:, :n], AF.Silu)

            # persistent scatter accumulator [DIM, NPC] (2 psum banks)
            fnT_acc = pacc.tile([DIM, NPC], f32)

            def mm2_block(lhsT, wmat, total, ve_tag, scale=None):
                """matmul lhsT.T @ wmat[:, :total] in 512-col psum chunks,
                ACT-evac (optionally scaled) into one f16 SBUF tile."""
                ve = wrk.tile([128, max(total, 512)], f16, tag=ve_tag)
                for (o2, n2) in _chunks(total, 512):
                    pw = pwp.tile([128, 512], f32, tag="pw")
                    nc.tensor.matmul(pw[:, :n2], lhsT, wmat[:, o2:o2 + n2],
                                     start=True, stop=True)
                    if scale is None:
                        nc.scalar.copy(ve[:, o2:o2 + n2], pw[:, :n2])
                    else:
                        nc.scalar.mul(ve[:, o2:o2 + n2], pw[:, :n2], scale)
                return ve


            # helper: in-place pairwise tree over innermost dim, then reduce
            def tree(P, nw, u, tag):
                """P: AP view [128, nw, u] (SBUF). Returns [128, nw] tile."""
                while u > 3 and u % 2 == 0:
                    h = u // 2
                    nc.vector.tensor_add(P[:, :, 0:h], P[:, :, 0:h], P[:, :, h:u])
                    u = h
                r = prd.tile([128, nw], f32, tag=tag + "_r")
                rv = r[:].rearrange("p (w o_) -> p w o_", o_=1)
                nc.vector.tensor_add(rv, P[:, :, 0:1], P[:, :, 1:2])
                if u == 3:
                    nc.vector.tensor_add(rv, rv, P[:, :, 2:3])
                return r

            # ---- phase 2: edge tiles ----
            for t in range(TILES):
                sl = slice(t * 128, (t + 1) * 128)
                sh0 = shp[:, 4 * t:4 * t + 1]
                sh1 = shp[:, 4 * t + 1:4 * t + 4]

                # o3 linears for the three cat sources -> one psum bank
                po3 = pmp.tile([128, 256], f32, tag="po3")  # 1 bank
                nc.tensor.matmul(po3[:, 0:80], fsrcT[:, sl], l1n[:], start=True, stop=True)
                nc.tensor.matmul(po3[:, 80:160], fdstT[:, sl], l1n[:], start=True, stop=True)
                nc.tensor.matmul(po3[:, 160:240], fedgeT[:, sl], l1e[:], start=True, stop=True)

                # assemble cat0 [128,96], cat1r [128,(3k,48u)]
                cat0 = wrk.tile([128, CAT0], f16, tag="cat0")
                cat1r = wrk.tile([128, 3 * CAT1], f16, tag="cat1r")
                c1v = cat1r[:].rearrange("p (k u) -> p k u", k=3)
                for j in range(3):
                    base = 80 * j
                    nc.scalar.copy(cat0[:, 32 * j:32 * (j + 1)], po3[:, base:base + 32])
                    src = po3[:, base + 32:base + 80].rearrange("p (u k) -> p k u", k=3)
                    nc.scalar.copy(c1v[:, :, 16 * j:16 * (j + 1)], src)

                # F2[u] = sum_k cat1[u,k]*sh1[k]  (vv input)
                f2a = wrk.tile([128, CAT1], f16, tag="f2a")
                f2 = wrk.tile([128, CAT1], f16, tag="f2")
                nc.vector.tensor_scalar_mul(f2a[:], cat1r[:, 0:48], sh1[:, 0:1])
                nc.vector.scalar_tensor_tensor(out=f2[:], in0=cat1r[:, 48:96],
                                               scalar=sh1[:, 1:2], in1=f2a[:],
                                               op0=OP.mult, op1=OP.add)
                nc.vector.scalar_tensor_tensor(out=f2[:], in0=cat1r[:, 96:144],
                                               scalar=sh1[:, 2:3], in1=f2[:],
                                               op0=OP.mult, op1=OP.add)

                # ---- mm2 + per-edge TP products ----
                # SS block: cols (32w x 96u); products scaled by sh0
                PA = prd.tile([128, (MUL0 + MUL1) * CAT0], f16, tag="PA")
                pav = PA[:].rearrange("p (g u) -> p g u", u=CAT0)
                ve_ss = mm2_block(hsb[:, sl], wss[:], MUL0 * CAT0, "ve_ss", scale=sh0)
                nc.vector.tensor_tensor(
                    out=pav[:, 0:MUL0, :],
                    in0=ve_ss[:, :MUL0 * CAT0].rearrange("p (w u) -> p w u", u=CAT0),
                    in1=cat0[:].rearrange("p (o_ u) -> p o_ u", o_=1)
                        .broadcast_to([128, MUL0, CAT0]),
                    op=OP.mult)

                # VV block: cols (32w x 48u); in1 = F2 (already has sh1 folded)
                PB = prd.tile([128, (MUL0 + 3 * MUL1) * CAT1], f16, tag="PB")
                pbv = PB[:].rearrange("p (g u) -> p g u", u=CAT1)
                ve_vv = mm2_block(hsb[:, sl], wvv[:], MUL0 * CAT1, "ve_vv")
                nc.vector.tensor_tensor(
                    out=pbv[:, 0:MUL0, :],
                    in0=ve_vv[:, :MUL0 * CAT1].rearrange("p (w u) -> p w u", u=CAT1),
                    in1=f2[:].rearrange("p (o_ u) -> p o_ u", o_=1)
                        .broadcast_to([128, MUL0, CAT1]),
                    op=OP.mult)

                # SV block: cols (16w x 96u); t16[w] = sum_u cat0[u]*w_sv
                ve_sv = mm2_block(hsb[:, sl], wsv[:], MUL1 * CAT0, "ve_sv")
                nc.vector.tensor_tensor(
                    out=pav[:, MUL0:MUL0 + MUL1, :],
                    in0=ve_sv[:, :MUL1 * CAT0].rearrange("p (w u) -> p w u", u=CAT0),
                    in1=cat0[:].rearrange("p (o_ u) -> p o_ u", o_=1)
                        .broadcast_to([128, MUL1, CAT0]),
                    op=OP.mult)

                # VS block: cols (16w x 48u); shared over k, scaled by sh0
                pvsv = pbv[:, MUL0:, :].rearrange("p (k w) u -> p k w u", k=3)
                ve_vs = mm2_block(hsb[:, sl], wvs[:], MUL1 * CAT1, "ve_vs", scale=sh0)
                iv = ve_vs[:, :MUL1 * CAT1].rearrange("p (w u) -> p w u", u=CAT1)
                for k in range(3):
                    bc = cat1r[:, k * CAT1:(k + 1) * CAT1] \
                        .rearrange("p (o_ u) -> p o_ u", o_=1).broadcast_to([128, MUL1, CAT1])
                    nc.vector.tensor_tensor(out=pvsv[:, k, :, :], in0=iv, in1=bc, op=OP.mult)
                # fused trees over PA (u=96) and PB (u=48)
                rA = tree(pav, MUL0 + MUL1, CAT0, "rA")          # [128, 48]: y0a | t16
                rB = tree(pbv, MUL0 + 3 * MUL1, CAT1, "rB")      # [128, 80]: y0b | vs48(k,w)
                y0a = rA[:, 0:MUL0]; t16v = rA[:, MUL0:MUL0 + MUL1]
                y0b = rB[:, 0:MUL0]; vs48v = rB[:, MUL0:MUL0 + 3 * MUL1]

                # y0 = silu(y0a + y0b) -> fe_gated[:, :32]
                fe_g = outp.tile([128, DIM], f32, tag="fe_g")
                y0 = wrk.tile([128, MUL0], f32, tag="y0")
                nc.vector.tensor_add(y0[:], y0a, y0b)
                nc.scalar.activation(fe_g[:, 0:MUL0], y0[:], AF.Silu)

                # y1[(w,k)] = t16[w]*sh1[k] + vs48[(k,w)]
                y1 = wrk.tile([128, 3 * MUL1], f32, tag="y1")
                y1v = y1[:].rearrange("p (w k) -> p w k", k=3)
                t16b = t16v.rearrange("p (w o_) -> p w o_", o_=1).broadcast_to([128, MUL1, 3])
                sh1b = sh1[:].rearrange("p (o_ k) -> p o_ k", o_=1).broadcast_to([128, MUL1, 3])
                nc.vector.tensor_tensor(out=y1v, in0=t16b, in1=sh1b, op=OP.mult)
                vsv = vs48v.rearrange("p (k w) -> p w k", k=3)
                nc.vector.tensor_add(y1v, y1v, vsv)

                # gate: sigmoid(|y1|) per vector
                sq = wrk.tile([128, 3 * MUL1], f32, tag="sq")
                nc.scalar.activation(sq[:], y1[:], AF.Square)
                n2 = wrk.tile([128, MUL1], f32, tag="n2")
                nc.vector.tensor_reduce(n2[:], sq[:].rearrange("p (w k) -> p w k", k=3),
                                        axis=mybir.AxisListType.X, op=OP.add)
                nrm = wrk.tile([128, MUL1], f32, tag="nrm")
                nc.scalar.activation(nrm[:], n2[:], AF.Sqrt)
                gsig = wrk.tile([128, MUL1], f32, tag="gsig")
                nc.scalar.activation(gsig[:], nrm[:], AF.Sigmoid)
                gb = gsig[:].rearrange("p (w o_) -> p w o_", o_=1).broadcast_to([128, MUL1, 3])
                fgv = fe_g[:, MUL0:DIM].rearrange("p (w k) -> p w k", k=3)
                nc.vector.tensor_tensor(out=fgv, in0=y1v, in1=gb, op=OP.mult)

                # ---- sc_edge (B-form) ----
                x0e = fedge_p[:, t * DIM:t * DIM + MUL0]
                x1r = wrk.tile([128, 3 * MUL1], f32, tag="x1r")
                nc.scalar.copy(
                    x1r[:].rearrange("p (k u) -> p k u", k=3),
                    fedge_p[:, t * DIM + MUL0:(t + 1) * DIM].rearrange("p (u k) -> p k u", k=3))

                Pb0 = prd.tile([128, MUL0 * MUL0], f32, tag="Pb0")
                for (o, n) in _chunks(MUL0 * MUL0, 512):
                    nw = n // MUL0
                    pw = pwp.tile([128, 512], f32, tag="pw")
                    nc.tensor.matmul(pw[:, :n], seT[:, sl], w0re[:, o:o + n],
                                     start=True, stop=True)
                    ov = Pb0[:, o:o + n].rearrange("p (w u) -> p w u", u=MUL0)
                    iv = pw[:, :n].rearrange("p (w u) -> p w u", u=MUL0)
                    bc = x0e.rearrange("p (o_ u) -> p o_ u", o_=1).broadcast_to([128, nw, MUL0])
                    nc.vector.tensor_tensor(out=ov, in0=iv, in1=bc, op=OP.mult)
                sc0 = tree(Pb0[:].rearrange("p (w u) -> p w u", u=MUL0), MUL0, MUL0, "sc0")

                Pb1 = prd.tile([128, 3 * MUL1 * MUL1], f32, tag="Pb1")
                pb1v = Pb1[:].rearrange("p (k w u) -> p k w u", k=3, u=MUL1)
                pw = pwp.tile([128, 512], f32, tag="pw")
                nc.tensor.matmul(pw[:, :MUL1 * MUL1], seT[:, sl], w1re[:],
                                 start=True, stop=True)
                iv = pw[:, :MUL1 * MUL1].rearrange("p (w u) -> p w u", u=MUL1)
                for k in range(3):
                    bc = x1r[:, k * MUL1:(k + 1) * MUL1] \
                        .rearrange("p (o_ u) -> p o_ u", o_=1).broadcast_to([128, MUL1, MUL1])
                    nc.vector.tensor_tensor(out=pb1v[:, k, :, :], in0=iv, in1=bc, op=OP.mult)
                sc1 = tree(Pb1[:].rearrange("p (g u) -> p g u", u=MUL1), 3 * MUL1, MUL1, "sc1")

                sc_e = outp.tile([128, DIM], f32, tag="sc_e")
                nc.scalar.copy(sc_e[:, 0:MUL0], sc0[:])
                nc.scalar.copy(
                    sc_e[:, MUL0:DIM].rearrange("p (w k) -> p w k", k=3),
                    sc1[:].rearrange("p (k w) -> p w k", k=3))

                # ---- scatter-add into fnT_acc via 0/1 matmul ----
                oh = ohp.tile([128, NPC], f32, tag="oh")
                nc.gpsimd.dma_start(oh[:], D["onehot"][t])
                nc.tensor.matmul(fnT_acc[:, 0:512], fe_g[:], oh[:, 0:512],
                                 start=(t == 0), stop=(t == TILES - 1), skip_group_check=True)
                nc.tensor.matmul(fnT_acc[:, 512:NPC], fe_g[:], oh[:, 512:NPC],
                                 start=(t == 0), stop=(t == TILES - 1), skip_group_check=True)

                # ---- fe output: (gate @ L2e + sc_e)^T ----
                ptp = pmp.tile([DIM, 384], f32, tag="ptp")  # 1 bank
                nc.tensor.transpose(ptp[:, 0:128], fe_g[:], ident[:])
                nc.tensor.transpose(ptp[:, 128:256], sc_e[:], ident[:])
                geT = wrk.tile([DIM, 128], f32, tag="geT")
                nc.scalar.copy(geT[:], ptp[:, 0:128])
                scT = wrk.tile([DIM, 128], f32, tag="scT")
                nc.scalar.copy(scT[:], ptp[:, 128:256])
                nc.tensor.matmul(ptp[:, 256:384], l2e[:], geT[:], start=True, stop=True)
                feT_t = outp.tile([DIM, 128], f32, tag="feT_t")
                nc.vector.tensor_add(feT_t[:], ptp[:, 256:384], scT[:])
                nc.gpsimd.dma_start(feT_out[:, sl], feT_t[:])

            # ---- phase 3: node outputs ----
            fnT_sb = cst.tile([DIM, NPC], f32, tag="fnT_sb")
            nc.scalar.mul(fnT_sb[:], fnT_acc[:], 1.0 / N_AVG)
            for nt in range(NT_N):
                nsl = slice(nt * 128, (nt + 1) * 128)
                pl2n = pmp.tile([128, 256], f32, tag="po3")  # 1 bank
                nc.tensor.matmul(pl2n[:, 0:DIM], fnT_sb[:, nsl], l2n[:], start=True, stop=True)

                x0n = fnode_p[:, nt * DIM:nt * DIM + MUL0]
                x1rn = wrk.tile([128, 3 * MUL1], f32, tag="x1r")
                nc.scalar.copy(
                    x1rn[:].rearrange("p (k u) -> p k u", k=3),
                    fnode_p[:, nt * DIM + MUL0:(nt + 1) * DIM].rearrange("p (u k) -> p k u", k=3))

                Pb0 = prd.tile([128, MUL0 * MUL0], f32, tag="Pb0")
                for (o, n) in _chunks(MUL0 * MUL0, 512):
                    nw = n // MUL0
                    pw = pwp.tile([128, 512], f32, tag="pw")
                    nc.tensor.matmul(pw[:, :n], nembT[:, nsl], w0rn[:, o:o + n],
                                     start=True, stop=True)
                    ov = Pb0[:, o:o + n].rearrange("p (w u) -> p w u", u=MUL0)
                    iv = pw[:, :n].rearrange("p (w u) -> p w u", u=MUL0)
                    bc = x0n.rearrange("p (o_ u) -> p o_ u", o_=1).broadcast_to([128, nw, MUL0])
                    nc.vector.tensor_tensor(out=ov, in0=iv, in1=bc, op=OP.mult)
                sc0 = tree(Pb0[:].rearrange("p (w u) -> p w u", u=MUL0), MUL0, MUL0, "sc0")

                Pb1 = prd.tile([128, 3 * MUL1 * MUL1], f32, tag="Pb1")
                pb1v = Pb1[:].rearrange("p (k w u) -> p k w u", k=3, u=MUL1)
                pw = pwp.tile([128, 512], f32, tag="pw")
                nc.tensor.matmul(pw[:, :MUL1 * MUL1], nembT[:, nsl], w1rn[:],
                                 start=True, stop=True)
                iv = pw[:, :MUL1 * MUL1].rearrange("p (w u) -> p w u", u=MUL1)
                for k in range(3):
                    bc = x1rn[:, k * MUL1:(k + 1) * MUL1] \
                        .rearrange("p (o_ u) -> p o_ u", o_=1).broadcast_to([128, MUL1, MUL1])
                    nc.vector.tensor_tensor(out=pb1v[:, k, :, :], in0=iv, in1=bc, op=OP.mult)
                sc1 = tree(Pb1[:].rearrange("p (g u) -> p g u", u=MUL1), 3 * MUL1, MUL1, "sc1")

                fn_t = outp.tile([128, DIM], f32, tag="fn_t")
                nc.vector.tensor_add(fn_t[:, 0:MUL0], pl2n[:, 0:MUL0], sc0[:])
                nc.vector.tensor_add(
                    fn_t[:, MUL0:DIM].rearrange("p (w k) -> p w k", k=3),
                    pl2n[:, MUL0:DIM].rearrange("p (w k) -> p w k", k=3),
                    sc1[:].rearrange("p (k w) -> p w k", k=3))
                nc.gpsimd.dma_start(fn_out[:, nt * DIM:(nt + 1) * DIM], fn_t[:])

    import bass_rust as _bass_rust
    _bass_rust.move_matmul_waits_to_ldweights(nc.m)
    _bass_rust.generate_event_semaphores(nc)
    return nc


def _get_program():
    if "nc" not in _PROG_CACHE:
        _PROG_CACHE["nc"] = _build_program()
    return _PROG_CACHE["nc"]


# ---------------- entry point -------------------------------------------------
def kernel(**inputs):
    inputs = {k: np.asarray(v) for k, v in inputs.items()}
    in_maps, meta = _host_prep(inputs)
    nc = _get_program()

    from concourse.bass_utils import run_bass_kernel_spmd
    res = run_bass_kernel_spmd(nc, in_maps, list(range(NCORES)))
    _PROG_CACHE["last_results"] = res

    fn = np.zeros((N_NODES, DIM), np.float32)
    fe = np.zeros((N_EDGES, DIM), np.float32)
    for i in range(NCORES):
        ids, ln, lo, width = meta[i]
        r = res.results[i]
        fn_my = r["fn_out"].reshape(128, NT_N, DIM).transpose(1, 0, 2).reshape(NPC, DIM)
        fn[lo:lo + width] = fn_my[:width]
        fe[ids] = r["feT_out"].T[:ln]
    return fn, fe


# revision 17
# speedup vs baseline: 1.0204x; 1.0001x over previous
"""Trainium2 Bass kernel for nn_E3Convolution (E3 equivariant convolution).

Strategy (8 NeuronCores, edge-parallel, zero collectives):
  - Host sorts edges by dst and cuts the sorted list at segment boundaries into
    8 contiguous shards -> each core owns a disjoint contiguous node range, so
    the scatter-add needs no cross-core reduction at all.
  - Host prep is strictly index manipulation / layout permutation / weight
    reshaping; every FLOP that scales with edges or nodes runs on device.
  - Per core: PE does all shared-weight matmuls (radial MLP, o3 linears,
    s@W pre-contractions, scatter via 0/1 matrices, transposes); DVE does the
    per-edge tensor-product contractions via stride-0-broadcast access
    patterns + in-place tree reductions; ACT does activations and strided
    copies.
"""

import sys

sys.path.insert(0, "/opt/trn_rl_repo")

import numpy as np

# ---------------- problem constants (hardcoded from the spec) ----------------
MUL0, MUL1 = 32, 16
DIM = MUL0 + 3 * MUL1            # 80
NUM_TYPE, BASIS, HIDDEN = 4, 32, 128
N_NODES, N_EDGES = 5000, 30000
N_AVG = 6.0
CAT0, CAT1 = 3 * MUL0, 3 * MUL1  # 96, 48
SQ3 = np.float32(np.sqrt(3.0))

NCORES = 8
EPC = 3840                        # padded edges per core (30 tiles of 128)
TILES = EPC // 128                # 30
NPC = 768                         # padded nodes per core (6 tiles of 128)
NT_N = NPC // 128                 # 6

F32 = None  # filled at import of mybir below


# ---------------- host-side preparation (index-only + weight reshape) --------
def _block_diag_L(L0, L1, mul0, mul1):
    """o3_linear as a single [DIM, DIM] matrix (scalars block + L1 (x) I3)."""
    d = mul0 + 3 * mul1
    M = np.zeros((d, d), np.float32)
    M[:mul0, :mul0] = L0 / np.sqrt(mul0)
    for u in range(mul1):
        for w in range(mul1):
            v = L1[u, w] / np.sqrt(mul1)
            for k in range(3):
                M[mul0 + 3 * u + k, mul0 + 3 * w + k] = v
    return M


def _reorder_wm2(Wm2):
    """Split+reorder Wm2 columns into per-path blocks with (w-outer, u-inner)
    column order, with all normalization constants folded in."""
    c = HIDDEN
    o = 0
    w_ss = Wm2[:, o:o + CAT0 * MUL0].reshape(c, CAT0, MUL0); o += CAT0 * MUL0
    w_sv = Wm2[:, o:o + CAT0 * MUL1].reshape(c, CAT0, MUL1); o += CAT0 * MUL1
    w_vs = Wm2[:, o:o + CAT1 * MUL1].reshape(c, CAT1, MUL1); o += CAT1 * MUL1
    w_vv = Wm2[:, o:].reshape(c, CAT1, MUL0)
    s = np.float32(1.0 / (np.sqrt(HIDDEN) * np.sqrt(CAT0 + CAT1)))
    # (w-outer, u-inner): dev[:, w*U + u] = w_path[:, u, w]
    dev_ss = np.ascontiguousarray(w_ss.transpose(0, 2, 1).reshape(c, -1)) * s
    dev_sv = np.ascontiguousarray(w_sv.transpose(0, 2, 1).reshape(c, -1)) * s
    dev_vs = np.ascontiguousarray(w_vs.transpose(0, 2, 1).reshape(c, -1)) * s
    dev_vv = np.ascontiguousarray(w_vv.transpose(0, 2, 1).reshape(c, -1)) * (s / SQ3)
    return dev_ss, dev_sv, dev_vs, dev_vv


def _reorder_sc(W0, W1, T):
    """sc_tp weights as [T, (w,u)] matrices for the B-form, norms folded."""
    s0 = np.float32(1.0 / np.sqrt(MUL0 * T))
    s1 = np.float32(1.0 / np.sqrt(MUL1 * T))
    # W0 [u, t, w] -> W0r[t, w*MUL0 + u]
    W0r = np.ascontiguousarray(W0.transpose(1, 2, 0).reshape(T, -1)) * s0
    W1r = np.ascontiguousarray(W1.transpose(1, 2, 0).reshape(T, -1)) * s1
    return W0r, W1r


def _shard_edges(edge_dst):
    """Sort edges by dst (stable) and cut at segment boundaries near i*E/8.
    Returns (sorted_ids, cuts[9], node_lo[9])."""
    order = np.argsort(edge_dst, kind="stable")
    dst_sorted = edge_dst[order]
    cuts = [0]
    for i in range(1, NCORES):
        t = (N_EDGES * i) // NCORES
        # move t forward to the next segment boundary (dst changes)
        while t < N_EDGES and t > 0 and dst_sorted[t] == dst_sorted[t - 1]:
            t += 1
        cuts.append(min(t, N_EDGES))
    cuts.append(N_EDGES)
    cuts = np.maximum.accumulate(np.asarray(cuts))
    node_lo = [0]
    for i in range(1, NCORES):
        a, b = cuts[i], cuts[i + 1]
        node_lo.append(int(dst_sorted[a]) if b > a else node_lo[-1])
    node_lo.append(N_NODES)
    return order, cuts, node_lo


def _host_prep(inputs):
    f_node = inputs["f_node"]; f_edge = inputs["f_edge"]
    sh = inputs["sh"]; node_emb = inputs["node_emb"]; length_emb = inputs["length_emb"]
    edge_src = inputs["edge_src"]; edge_dst = inputs["edge_dst"]

    order, cuts, node_lo = _shard_edges(edge_dst)

    # replicated (weight) tensors
    wss, wsv, wvs, wvv = _reorder_wm2(inputs["Wm2"])
    rep = {
        "Wm1s": np.ascontiguousarray(inputs["Wm1"] * np.float32(1.0 / np.sqrt(BASIS))),
        "Wss": wss, "Wsv": wsv, "Wvs": wvs, "Wvv": wvv,
        "L1n": _block_diag_L(inputs["L1n0"], inputs["L1n1"], MUL0, MUL1),
        "L1e": _block_diag_L(inputs["L1e0"], inputs["L1e1"], MUL0, MUL1),
        "L2n": _block_diag_L(inputs["L2n0"], inputs["L2n1"], MUL0, MUL1),
        "L2e": _block_diag_L(inputs["L2e0"], inputs["L2e1"], MUL0, MUL1),
    }
    rep["W0re"], rep["W1re"] = _reorder_sc(inputs["W_sce0"], inputs["W_sce1"], 2 * NUM_TYPE + BASIS)
    rep["W0rn"], rep["W1rn"] = _reorder_sc(inputs["W_scn0"], inputs["W_scn1"], NUM_TYPE)

    in_maps = []
    meta = []
    for i in range(NCORES):
        a, b = int(cuts[i]), int(cuts[i + 1])
        ln = b - a
        assert ln <= EPC, f"shard {i} too long: {ln}"
        ids = order[a:b]
        lo, hi = node_lo[i], node_lo[i + 1]
        width = hi - lo
        assert width <= NPC, f"node range {i} too wide: {width}"

        src = edge_src[ids]; dst = edge_dst[ids]

        def padE(x, fill=0.0):
            out = np.zeros((EPC,) + x.shape[1:], np.float32)
            out[:ln] = x
            return out

        fsrc = padE(f_node[src]); fdst = padE(f_node[dst]); fedg = padE(f_edge[ids])
        le = padE(length_emb[ids]); shp = padE(sh[ids])
        se = padE(np.concatenate([node_emb[src], node_emb[dst], length_emb[ids]], axis=-1))

        onehot = np.zeros((EPC, NPC), np.float32)
        onehot[np.arange(ln), dst - lo] = 1.0

        fnode_my = np.zeros((NPC, DIM), np.float32); fnode_my[:width] = f_node[lo:hi]
        nemb_my = np.zeros((NPC, NUM_TYPE), np.float32); nemb_my[:width] = node_emb[lo:hi]

        m = {
            "fsrcT": np.ascontiguousarray(fsrc.T).astype(np.float16),
            "fdstT": np.ascontiguousarray(fdst.T).astype(np.float16),
            "fedgeT": np.ascontiguousarray(fedg.T).astype(np.float16),
            "leT": np.ascontiguousarray(le.T).astype(np.float16),
            "seT": np.ascontiguousarray(se.T).astype(np.float16),
            # e-on-partition layouts [128, TILES, X]
            "fedge_p": np.ascontiguousarray(
                fedg.reshape(TILES, 128, DIM).transpose(1, 0, 2)
                .reshape(128, TILES * DIM)).astype(np.float16),
            "shp": np.ascontiguousarray(
                shp.reshape(TILES, 128, 4).transpose(1, 0, 2).reshape(128, TILES * 4)),
            "onehot": np.ascontiguousarray(onehot.reshape(TILES, 128, NPC)),
            "fnode_p": np.ascontiguousarray(
                fnode_my.reshape(NT_N, 128, DIM).transpose(1, 0, 2)
                .reshape(128, NT_N * DIM)).astype(np.float16),
            "nembT": np.ascontiguousarray(nemb_my.T).astype(np.float16),
        }
        m.update({k: (v.astype(np.float16) if k not in ("L2n", "L2e") else v)
                  for k, v in rep.items()})
        in_maps.append(m)
        meta.append((ids, ln, lo, width))
    return in_maps, meta


# ---------------- device program ---------------------------------------------
_PROG_CACHE = {}


def _chunks(total, size):
    out = []
    o = 0
    while o < total:
        c = min(size, total - o)
        out.append((o, c))
        o += c
    return out


def _build_program():
    from concourse import bass, mybir
    from concourse.tile import TileContext
    from concourse.masks import make_identity

    f32 = mybir.dt.float32
    f16 = mybir.dt.float16
    AF = mybir.ActivationFunctionType
    OP = mybir.AluOpType

    nc = bass.Bass()

    # ---- DRAM I/O ----
    F16_INPUTS = {"fsrcT", "fdstT", "fedgeT", "leT", "seT", "fedge_p", "fnode_p",
                  "nembT", "Wm1s", "Wss", "Wsv", "Wvs", "Wvv", "L1n", "L1e",
                  "W0re", "W1re", "W0rn", "W1rn"}
    D = {}
    def din(name, shape):
        dt = f16 if name in F16_INPUTS else f32
        D[name] = nc.dram_tensor(name, list(shape), dt, kind="ExternalInput")
    for nm, shp in [
        ("fsrcT", (DIM, EPC)), ("fdstT", (DIM, EPC)), ("fedgeT", (DIM, EPC)),
        ("leT", (BASIS, EPC)), ("seT", (2 * NUM_TYPE + BASIS, EPC)),
        ("fedge_p", (128, TILES * DIM)), ("shp", (128, TILES * 4)),
        ("onehot", (TILES, 128, NPC)),
        ("fnode_p", (128, NT_N * DIM)), ("nembT", (NUM_TYPE, NPC)),
        ("Wm1s", (BASIS, HIDDEN)),
        ("Wss", (HIDDEN, MUL0 * CAT0)), ("Wsv", (HIDDEN, MUL1 * CAT0)),
        ("Wvs", (HIDDEN, MUL1 * CAT1)), ("Wvv", (HIDDEN, MUL0 * CAT1)),
        ("L1n", (DIM, DIM)), ("L1e", (DIM, DIM)), ("L2n", (DIM, DIM)), ("L2e", (DIM, DIM)),
        ("W0re", (2 * NUM_TYPE + BASIS, MUL0 * MUL0)),
        ("W1re", (2 * NUM_TYPE + BASIS, MUL1 * MUL1)),
        ("W0rn", (NUM_TYPE, MUL0 * MUL0)), ("W1rn", (NUM_TYPE, MUL1 * MUL1)),
    ]:
        din(nm, shp)
    feT_out = nc.dram_tensor("feT_out", [DIM, EPC], f32, kind="ExternalOutput")
    fn_out = nc.dram_tensor("fn_out", [128, NT_N * DIM], f32, kind="ExternalOutput")

    with TileContext(nc) as tc:
        with (
            tc.tile_pool(name="const", bufs=1) as cst,
            tc.tile_pool(name="work", bufs=3) as wrk,
            tc.tile_pool(name="prod", bufs=2) as prd,
            tc.tile_pool(name="oh", bufs=5) as ohp,
            tc.tile_pool(name="outp", bufs=6) as outp,
            tc.tile_pool(name="pacc", bufs=1, space="PSUM") as pacc,
            tc.tile_pool(name="pw", bufs=4, space="PSUM") as pwp,
            tc.tile_pool(name="pmisc", bufs=1, space="PSUM") as pmp,
        ):
            # ---- load constants / per-core resident arrays ----
            def load(name, split=1):
                t = cst.tile(list(D[name].shape), D[name].dtype, tag=name)
                cols = D[name].shape[-1]
                step = (cols + split - 1) // split
                for (o, n) in _chunks(cols, step):
                    nc.gpsimd.dma_start(t[:, o:o + n], D[name][:, o:o + n])
                return t
            ident = cst.tile([128, 128], f32, tag="ident")
            make_identity(nc, ident[:])
            wm1 = load("Wm1s"); leT = load("leT", split=4)
            fsrcT = load("fsrcT", split=4); fdstT = load("fdstT", split=4)
            fedgeT = load("fedgeT", split=4)
            l1n = load("L1n"); l1e = load("L1e"); shp = load("shp")
            wss = load("Wss", split=2); wvv = load("Wvv"); wsv = load("Wsv")
            wvs = load("Wvs")
            seT = load("seT", split=4); fedge_p = load("fedge_p", split=4)
            w0re = load("W0re"); w1re = load("W1re")
            l2n = load("L2n"); l2e = load("L2e")
            w0rn = load("W0rn"); w1rn = load("W1rn")
            fnode_p = load("fnode_p"); nembT = load("nembT")

            # ---- phase 1: hT = silu(Wm1s.T @ leT)  [HIDDEN, EPC] ----
            hsb = cst.tile([128, EPC], f16, tag="hsb")
            for (o, n) in _chunks(EPC, 512):
                ph = pwp.tile([128, 512], f32, tag="pw")
                nc.tensor.matmul(ph[:, :n], wm1[:], leT[:, o:o + n], start=True, stop=True)
                nc.scalar.activation(hsb[:, o:o + n], ph[:, :n], AF.Silu)

            # persistent scatter accumulator [DIM, NPC] (2 psum banks)
            fnT_acc = pacc.tile([DIM, NPC], f32)

            def mm2_block(lhsT, wmat, total, ve_tag, scale=None):
                """matmul lhsT.T @ wmat[:, :total] in 512-col psum chunks,
                ACT-evac (optionally scaled) into one f16 SBUF tile."""
                ve = wrk.tile([128, max(total, 512)], f16, tag=ve_tag)
                for (o2, n2) in _chunks(total, 512):
                    pw = pwp.tile([128, 512], f32, tag="pw")
                    nc.tensor.matmul(pw[:, :n2], lhsT, wmat[:, o2:o2 + n2],
                                     start=True, stop=True)
                    if scale is None:
                        nc.scalar.copy(ve[:, o2:o2 + n2], pw[:, :n2])
                    else:
                        nc.scalar.mul(ve[:, o2:o2 + n2], pw[:, :n2], scale)
                return ve


            # helper: in-place pairwise tree over innermost dim, then reduce
            def tree(P, nw, u, tag):
                """P: AP view [128, nw, u] (SBUF). Returns [128, nw] tile."""
                while u > 3 and u % 2 == 0:
                    h = u // 2
                    nc.vector.tensor_add(P[:, :, 0:h], P[:, :, 0:h], P[:, :, h:u])
                    u = h
                r = prd.tile([128, nw], f32, tag=tag + "_r")
                rv = r[:].rearrange("p (w o_) -> p w o_", o_=1)
                nc.vector.tensor_add(rv, P[:, :, 0:1], P[:, :, 1:2])
                if u == 3:
                    nc.vector.tensor_add(rv, rv, P[:, :, 2:3])
                return r

            # ---- phase 2: edge tiles ----
            for t in range(TILES):
                sl = slice(t * 128, (t + 1) * 128)
                sh0 = shp[:, 4 * t:4 * t + 1]
                sh1 = shp[:, 4 * t + 1:4 * t + 4]

                # o3 linears for the three cat sources -> one psum bank
                po3 = pmp.tile([128, 256], f32, tag="po3")  # 1 bank
                nc.tensor.matmul(po3[:, 0:80], fsrcT[:, sl], l1n[:], start=True, stop=True)
                nc.tensor.matmul(po3[:, 80:160], fdstT[:, sl], l1n[:], start=True, stop=True)
                nc.tensor.matmul(po3[:, 160:240], fedgeT[:, sl], l1e[:], start=True, stop=True)

                # assemble cat0 [128,96], cat1r [128,(3k,48u)]
                cat0 = wrk.tile([128, CAT0], f16, tag="cat0")
                cat1r = wrk.tile([128, 3 * CAT1], f16, tag="cat1r")
                c1v = cat1r[:].rearrange("p (k u) -> p k u", k=3)
                for j in range(3):
                    base = 80 * j
                    nc.scalar.copy(cat0[:, 32 * j:32 * (j + 1)], po3[:, base:base + 32])
                    src = po3[:, base + 32:base + 80].rearrange("p (u k) -> p k u", k=3)
                    nc.scalar.copy(c1v[:, :, 16 * j:16 * (j + 1)], src)

                # F2[u] = sum_k cat1[u,k]*sh1[k]  (vv input)
                f2a = wrk.tile([128, CAT1], f16, tag="f2a")
                f2 = wrk.tile([128, CAT1], f16, tag="f2")
                nc.vector.tensor_scalar_mul(f2a[:], cat1r[:, 0:48], sh1[:, 0:1])
                nc.vector.scalar_tensor_tensor(out=f2[:], in0=cat1r[:, 48:96],
                                               scalar=sh1[:, 1:2], in1=f2a[:],
                                               op0=OP.mult, op1=OP.add)
                nc.vector.scalar_tensor_tensor(out=f2[:], in0=cat1r[:, 96:144],
                                               scalar=sh1[:, 2:3], in1=f2[:],
                                               op0=OP.mult, op1=OP.add)

                # ---- mm2 + per-edge TP products ----
                # SS block: cols (32w x 96u); products scaled by sh0
                PA = prd.tile([128, (MUL0 + MUL1) * CAT0], f16, tag="PA")
                pav = PA[:].rearrange("p (g u) -> p g u", u=CAT0)
                ve_ss = mm2_block(hsb[:, sl], wss[:], MUL0 * CAT0, "ve_ss", scale=sh0)
                nc.vector.tensor_tensor(
                    out=pav[:, 0:MUL0, :],
                    in0=ve_ss[:, :MUL0 * CAT0].rearrange("p (w u) -> p w u", u=CAT0),
                    in1=cat0[:].rearrange("p (o_ u) -> p o_ u", o_=1)
                        .broadcast_to([128, MUL0, CAT0]),
                    op=OP.mult)

                # VV block: cols (32w x 48u); in1 = F2 (already has sh1 folded)
                PB = prd.tile([128, (MUL0 + 3 * MUL1) * CAT1], f16, tag="PB")
                pbv = PB[:].rearrange("p (g u) -> p g u", u=CAT1)
                ve_vv = mm2_block(hsb[:, sl], wvv[:], MUL0 * CAT1, "ve_vv")
                nc.vector.tensor_tensor(
                    out=pbv[:, 0:MUL0, :],
                    in0=ve_vv[:, :MUL0 * CAT1].rearrange("p (w u) -> p w u", u=CAT1),
                    in1=f2[:].rearrange("p (o_ u) -> p o_ u", o_=1)
                        .broadcast_to([128, MUL0, CAT1]),
                    op=OP.mult)

                # SV block: cols (16w x 96u); t16[w] = sum_u cat0[u]*w_sv
                ve_sv = mm2_block(hsb[:, sl], wsv[:], MUL1 * CAT0, "ve_sv")
                nc.vector.tensor_tensor(
                    out=pav[:, MUL0:MUL0 + MUL1, :],
                    in0=ve_sv[:, :MUL1 * CAT0].rearrange("p (w u) -> p w u", u=CAT0),
                    in1=cat0[:].rearrange("p (o_ u) -> p o_ u", o_=1)
                        .broadcast_to([128, MUL1, CAT0]),
                    op=OP.mult)

                # VS block: cols (16w x 48u); shared over k, scaled by sh0
                pvsv = pbv[:, MUL0:, :].rearrange("p (k w) u -> p k w u", k=3)
                ve_vs = mm2_block(hsb[:, sl], wvs[:], MUL1 * CAT1, "ve_vs", scale=sh0)
                iv = ve_vs[:, :MUL1 * CAT1].rearrange("p (w u) -> p w u", u=CAT1)
                for k in range(3):
                    bc = cat1r[:, k * CAT1:(k + 1) * CAT1] \
                        .rearrange("p (o_ u) -> p o_ u", o_=1).broadcast_to([128, MUL1, CAT1])
                    nc.vector.tensor_tensor(out=pvsv[:, k, :, :], in0=iv, in1=bc, op=OP.mult)
                # fused trees over PA (u=96) and PB (u=48)
                rA = tree(pav, MUL0 + MUL1, CAT0, "rA")          # [128, 48]: y0a | t16
                rB = tree(pbv, MUL0 + 3 * MUL1, CAT1, "rB")      # [128, 80]: y0b | vs48(k,w)
                y0a = rA[:, 0:MUL0]; t16v = rA[:, MUL0:MUL0 + MUL1]
                y0b = rB[:, 0:MUL0]; vs48v = rB[:, MUL0:MUL0 + 3 * MUL1]

                # y0 = silu(y0a + y0b) -> fe_gated[:, :32]
                fe_g = outp.tile([128, DIM], f32, tag="fe_g")
                y0 = wrk.tile([128, MUL0], f32, tag="y0")
                nc.vector.tensor_add(y0[:], y0a, y0b)
                nc.scalar.activation(fe_g[:, 0:MUL0], y0[:], AF.Silu)

                # y1[(w,k)] = t16[w]*sh1[k] + vs48[(k,w)]
                y1 = wrk.tile([128, 3 * MUL1], f32, tag="y1")
                y1v = y1[:].rearrange("p (w k) -> p w k", k=3)
                t16b = t16v.rearrange("p (w o_) -> p w o_", o_=1).broadcast_to([128, MUL1, 3])
                sh1b = sh1[:].rearrange("p (o_ k) -> p o_ k", o_=1).broadcast_to([128, MUL1, 3])
                nc.vector.tensor_tensor(out=y1v, in0=t16b, in1=sh1b, op=OP.mult)
                vsv = vs48v.rearrange("p (k w) -> p w k", k=3)
                nc.vector.tensor_add(y1v, y1v, vsv)

                # gate: sigmoid(|y1|) per vector
                sq = wrk.tile([128, 3 * MUL1], f32, tag="sq")
                nc.scalar.activation(sq[:], y1[:], AF.Square)
                n2 = wrk.tile([128, MUL1], f32, tag="n2")
                nc.vector.tensor_reduce(n2[:], sq[:].rearrange("p (w k) -> p w k", k=3),
                                        axis=mybir.AxisListType.X, op=OP.add)
                nrm = wrk.tile([128, MUL1], f32, tag="nrm")
                nc.scalar.activation(nrm[:], n2[:], AF.Sqrt)
                gsig = wrk.tile([128, MUL1], f32, tag="gsig")
                nc.scalar.activation(gsig[:], nrm[:], AF.Sigmoid)
                gb = gsig[:].rearrange("p (w o_) -> p w o_", o_=1).broadcast_to([128, MUL1, 3])
                fgv = fe_g[:, MUL0:DIM].rearrange("p (w k) -> p w k", k=3)
                nc.vector.tensor_tensor(out=fgv, in0=y1v, in1=gb, op=OP.mult)

                # ---- sc_edge (B-form) ----
                x0e = fedge_p[:, t * DIM:t * DIM + MUL0]
                x1r = wrk.tile([128, 3 * MUL1], f32, tag="x1r")
                nc.scalar.copy(
                    x1r[:].rearrange("p (k u) -> p k u", k=3),
                    fedge_p[:, t * DIM + MUL0:(t + 1) * DIM].rearrange("p (u k) -> p k u", k=3))

                Pb0 = prd.tile([128, MUL0 * MUL0], f32, tag="Pb0")
                for (o, n) in _chunks(MUL0 * MUL0, 512):
                    nw = n // MUL0
                    pw = pwp.tile([128, 512], f32, tag="pw")
                    nc.tensor.matmul(pw[:, :n], seT[:, sl], w0re[:, o:o + n],
                                     start=True, stop=True)
                    ov = Pb0[:, o:o + n].rearrange("p (w u) -> p w u", u=MUL0)
                    iv = pw[:, :n].rearrange("p (w u) -> p w u", u=MUL0)
                    bc = x0e.rearrange("p (o_ u) -> p o_ u", o_=1).broadcast_to([128, nw, MUL0])
                    nc.vector.tensor_tensor(out=ov, in0=iv, in1=bc, op=OP.mult)
                sc0 = tree(Pb0[:].rearrange("p (w u) -> p w u", u=MUL0), MUL0, MUL0, "sc0")

                Pb1 = prd.tile([128, 3 * MUL1 * MUL1], f32, tag="Pb1")
                pb1v = Pb1[:].rearrange("p (k w u) -> p k w u", k=3, u=MUL1)
                pw = pwp.tile([128, 512], f32, tag="pw")
                nc.tensor.matmul(pw[:, :MUL1 * MUL1], seT[:, sl], w1re[:],
                                 start=True, stop=True)
                iv = pw[:, :MUL1 * MUL1].rearrange("p (w u) -> p w u", u=MUL1)
                for k in range(3):
                    bc = x1r[:, k * MUL1:(k + 1) * MUL1] \
                        .rearrange("p (o_ u) -> p o_ u", o_=1).broadcast_to([128, MUL1, MUL1])
                    nc.vector.tensor_tensor(out=pb1v[:, k, :, :], in0=iv, in1=bc, op=OP.mult)
                sc1 = tree(Pb1[:].rearrange("p (g u) -> p g u", u=MUL1), 3 * MUL1, MUL1, "sc1")

                sc_e = outp.tile([128, DIM], f32, tag="sc_e")
                nc.scalar.copy(sc_e[:, 0:MUL0], sc0[:])
                nc.scalar.copy(
                    sc_e[:, MUL0:DIM].rearrange("p (w k) -> p w k", k=3),
                    sc1[:].rearrange("p (k w) -> p w k", k=3))

                # ---- scatter-add into fnT_acc via 0/1 matmul ----
                oh = ohp.tile([128, NPC], f32, tag="oh")
                nc.gpsimd.dma_start(oh[:], D["onehot"][t])
                nc.tensor.matmul(fnT_acc[:, 0:512], fe_g[:], oh[:, 0:512],
                                 start=(t == 0), stop=(t == TILES - 1), skip_group_check=True)
                nc.tensor.matmul(fnT_acc[:, 512:NPC], fe_g[:], oh[:, 512:NPC],
                                 start=(t == 0), stop=(t == TILES - 1), skip_group_check=True)

                # ---- fe output: (gate @ L2e + sc_e)^T ----
                ptp = pmp.tile([DIM, 384], f32, tag="ptp")  # 1 bank
                nc.tensor.transpose(ptp[:, 0:128], fe_g[:], ident[:])
                nc.tensor.transpose(ptp[:, 128:256], sc_e[:], ident[:])
                geT = wrk.tile([DIM, 128], f32, tag="geT")
                nc.scalar.copy(geT[:], ptp[:, 0:128])
                scT = wrk.tile([DIM, 128], f32, tag="scT")
                nc.scalar.copy(scT[:], ptp[:, 128:256])
                nc.tensor.matmul(ptp[:, 256:384], l2e[:], geT[:], start=True, stop=True)
                feT_t = outp.tile([DIM, 128], f32, tag="feT_t")
                nc.vector.tensor_add(feT_t[:], ptp[:, 256:384], scT[:])
                nc.gpsimd.dma_start(feT_out[:, sl], feT_t[:])

            # ---- phase 3: node outputs ----
            fnT_sb = cst.tile([DIM, NPC], f32, tag="fnT_sb")
            nc.scalar.mul(fnT_sb[:], fnT_acc[:], 1.0 / N_AVG)
            for nt in range(NT_N):
                nsl = slice(nt * 128, (nt + 1) * 128)
                pl2n = pmp.tile([128, 256], f32, tag="po3")  # 1 bank
                nc.tensor.matmul(pl2n[:, 0:DIM], fnT_sb[:, nsl], l2n[:], start=True, stop=True)

                x0n = fnode_p[:, nt * DIM:nt * DIM + MUL0]
                x1rn = wrk.tile([128, 3 * MUL1], f32, tag="x1r")
                nc.scalar.copy(
                    x1rn[:].rearrange("p (k u) -> p k u", k=3),
                    fnode_p[:, nt * DIM + MUL0:(nt + 1) * DIM].rearrange("p (u k) -> p k u", k=3))

                Pb0 = prd.tile([128, MUL0 * MUL0], f32, tag="Pb0")
                for (o, n) in _chunks(MUL0 * MUL0, 512):
                    nw = n // MUL0
                    pw = pwp.tile([128, 512], f32, tag="pw")
                    nc.tensor.matmul(pw[:, :n], nembT[:, nsl], w0rn[:, o:o + n],
                                     start=True, stop=True)
                    ov = Pb0[:, o:o + n].rearrange("p (w u) -> p w u", u=MUL0)
                    iv = pw[:, :n].rearrange("p (w u) -> p w u", u=MUL0)
                    bc = x0n.rearrange("p (o_ u) -> p o_ u", o_=1).broadcast_to([128, nw, MUL0])
                    nc.vector.tensor_tensor(out=ov, in0=iv, in1=bc, op=OP.mult)
                sc0 = tree(Pb0[:].rearrange("p (w u) -> p w u", u=MUL0), MUL0, MUL0, "sc0")

                Pb1 = prd.tile([128, 3 * MUL1 * MUL1], f32, tag="Pb1")
                pb1v = Pb1[:].rearrange("p (k w u) -> p k w u", k=3, u=MUL1)
                pw = pwp.tile([128, 512], f32, tag="pw")
                nc.tensor.matmul(pw[:, :MUL1 * MUL1], nembT[:, nsl], w1rn[:],
                                 start=True, stop=True)
                iv = pw[:, :MUL1 * MUL1].rearrange("p (w u) -> p w u", u=MUL1)
                for k in range(3):
                    bc = x1rn[:, k * MUL1:(k + 1) * MUL1] \
                        .rearrange("p (o_ u) -> p o_ u", o_=1).broadcast_to([128, MUL1, MUL1])
                    nc.vector.tensor_tensor(out=pb1v[:, k, :, :], in0=iv, in1=bc, op=OP.mult)
                sc1 = tree(Pb1[:].rearrange("p (g u) -> p g u", u=MUL1), 3 * MUL1, MUL1, "sc1")

                fn_t = outp.tile([128, DIM], f32, tag="fn_t")
                nc.vector.tensor_add(fn_t[:, 0:MUL0], pl2n[:, 0:MUL0], sc0[:])
                nc.vector.tensor_add(
                    fn_t[:, MUL0:DIM].rearrange("p (w k) -> p w k", k=3),
                    pl2n[:, MUL0:DIM].rearrange("p (w k) -> p w k", k=3),
                    sc1[:].rearrange("p (k w) -> p w k", k=3))
                nc.gpsimd.dma_start(fn_out[:, nt * DIM:(nt + 1) * DIM], fn_t[:])

    import bass_rust as _bass_rust
    _bass_rust.move_matmul_waits_to_ldweights(nc.m)
    _bass_rust.generate_event_semaphores(nc)
    return nc


def _get_program():
    if "nc" not in _PROG_CACHE:
        _PROG_CACHE["nc"] = _build_program()
    return _PROG_CACHE["nc"]


# ---------------- entry point -------------------------------------------------
def kernel(**inputs):
    inputs = {k: np.asarray(v) for k, v in inputs.items()}
    in_maps, meta = _host_prep(inputs)
    nc = _get_program()

    from concourse.bass_utils import run_bass_kernel_spmd
    res = run_bass_kernel_spmd(nc, in_maps, list(range(NCORES)))
    _PROG_CACHE["last_results"] = res

    fn = np.zeros((N_NODES, DIM), np.float32)
    fe = np.zeros((N_EDGES, DIM), np.float32)
    for i in range(NCORES):
        ids, ln, lo, width = meta[i]
        r = res.results[i]
        fn_my = r["fn_out"].reshape(128, NT_N, DIM).transpose(1, 0, 2).reshape(NPC, DIM)
        fn[lo:lo + width] = fn_my[:width]
        fe[ids] = r["feT_out"].T[:ln]
    return fn, fe
